# revision 10
# baseline (speedup 1.0000x reference)
"""Trainium2 Bass kernel for the DART masked-MLP + log-semiring chain model.

Computes, for B=8192 samples distributed over 8 NeuronCores (1024 each):
  h1 = relu(x @ (m0*W0).T + b0)
  h2 = relu(h1 @ (m1*W1).T + b1)
  h3 = relu(h2 @ (m2*W2).T + b2)
  theta = (h3 @ (m3*W3).T + b3) -> (B, 128, 2, 4, 4) = (mu, alpha)
  logp  = -0.5*((x - mu)*exp(-alpha))**2 - alpha - 0.5*log(2pi) - log(4)
  out   = logexpmm(first, logexpmm(chain(inner), last))   # (B, 1, 1)

Device strategy (per core):
  - MADE masks premultiplied into the weights host-side; hidden units sorted
    by MADE degree so the masked weights are block lower triangular and ~47%
    of K-chunks are skipped.
  - ALL four matmul layers run fp8-e4m3 DoubleRow (256-wide contraction per
    instruction, 0.5 PE cycles per output column).  x is pre-transposed and
    pre-packed to the fp8 pair layout host-side, so no on-device transpose.
  - ALL biases are added in PSUM by tiny fp8 DoubleRow matmuls (stationary
    carries the bias row at 2^13, moving a constant 2^(S-13) row), freeing
    the relu from the per-partition-bias ACT so it can run on any engine.
  - The PSUM->fp8 relu+rescale is load-balanced across the ACT, Pool and DVE
    engines (single tensor_scalar op: max(psum*2^-S, 0)).
  - logp pipeline per 1024-wide theta pair-chunk: exp on ACT, the (mu-x)
    STT on DVE, tt/sq/P-store multiplies on Pool.  P is stored in fp16,
    pre-scaled by 8 (P<=0.106 for this model, so 8*P<1; the product picks up
    8^128 which is subtracted as a compile-time constant at the end).
  - The 126-step log-semiring chain runs in fp16 in the linear domain,
    folded simultaneously from both ends as paired steps in single DVE ops
    (fp16 gets the DVE 2x perf mode on the multiply).  A max-renormalization
    every 4 pair-steps keeps fp16 in range; its scale is applied one step
    late so the max/reciprocal run off the critical path.  L4 output chunks
    are produced outside-in so the chain overlaps the matmuls.
"""

import math

import numpy as np
import ml_dtypes

I = 128          # input size / positions
H = 2048         # hidden
A = 4            # alpha_dim
K = 2 * A * A    # 32 theta entries per position
B = 8192
NCORES = 8
BL = B // NCORES          # 1024 samples per core
NG = BL // 128            # 8 sample groups of 128
NK = H // 128             # 16 hidden chunks
NQ = (I * K) // 512       # 8 output q-chunks (512 wide = 16 positions)
C0 = 0.5 * math.log(2.0 * math.pi) + math.log(4.0)
SW = 13                   # weight scale 2^SW for fp8 (max |w|*2^13 ~ 181 < 240)
SA = 5                    # activation scale 2^SA for fp8 (max h*32 ~ 80 < 240)
SX = 5                    # x scale 2^SX for fp8 (max |x|*32 ~ 144 < 240)
SW0 = 11                  # W0 scale 2^SW0 (max |w0|*2^11 ~ 181 < 240)
SB = 13                   # bias fp8 scale 2^SB (max |b|*2^13 ~ 181 < 240)
LNC = 3.0 * math.log(2.0)   # P pre-scale ln(8); P*8 <= 0.85 < 1
# per-layer relu engine split: 'A' = ACT, 'D' = DVE.  (Pool/GPSIMD cannot
# read PSUM on TRN2, so it only gets SBUF-side work.)
RELU_ENG = ['A'] * 16

_bf16 = ml_dtypes.bfloat16
_f8 = ml_dtypes.float8_e4m3


def _make_meta():
    """Degree sort + triangular chunk metadata (static)."""
    hdeg = np.arange(H) % (I - 1)
    perm = np.argsort(hdeg, kind="stable")
    sdeg = hdeg[perm]
    km_l = []
    for m in range(NK):
        dhi = sdeg[128 * m + 127]
        km_l.append(max(k for k in range(NK) if sdeg[128 * k] <= dhi))
    km_4 = []
    for qc in range(NQ):
        dhi = 16 * qc + 15 - 1
        cands = [k for k in range(NK) if sdeg[128 * k] <= dhi]
        km_4.append(max(cands) if cands else -1)
    return perm, km_l, km_4


_PERM, _KM_L, _KM_4 = _make_meta()
_NP_L = [k // 2 + 1 for k in _KM_L]     # fp8 DoubleRow pair-chunks (256-wide K)
_NP_4 = [k // 2 + 1 for k in _KM_4]
_OFF1 = np.cumsum([0] + [p * 256 for p in _NP_L]).tolist()
_OFF3 = np.cumsum([0] + [p * 1024 for p in _NP_4]).tolist()


def _pack_pair_f8(WT, npairs, out_w, col_starts, scale):
    """fp8 DoubleRow blocks [128, 2, out_w] per (block, pair)."""
    cols = []
    for blk, (np_, c0) in enumerate(zip(npairs, col_starts)):
        for kp in range(np_):
            blkdat = np.stack(
                [WT[256 * kp + 128 * par:256 * kp + 128 * (par + 1),
                    c0:c0 + out_w] for par in range(2)], axis=1)
            cols.append(blkdat.reshape(128, 2 * out_w))
    arr = np.concatenate(cols, axis=1) * float(scale)
    assert np.abs(arr).max() < 235.0, np.abs(arr).max()
    return np.ascontiguousarray(arr).astype(_f8)


def _prep_inputs(x, W0, b0, W1, b1, W2, b2, W3, b3):
    """Host-side: premask, degree-sort, pack and cast the weights."""
    inp = np.arange(I)
    degrees = [inp] + [np.arange(H) % (I - 1) for _ in range(3)] + [np.arange(I) - 1]
    masks = [
        (d1[:, None] >= d0[None, :]).astype(np.float32)
        for d0, d1 in zip(degrees[:-1], degrees[1:])
    ]
    masks[-1] = np.repeat(masks[-1], K, axis=0)

    p = _PERM
    W0s = (masks[0] * W0)[p]
    b0s = b0[p]
    W1s = (masks[1] * W1)[p][:, p]
    b1s = b1[p]
    W2s = (masks[2] * W2)[p][:, p]
    b2s = b2[p]
    W3s = (masks[3] * W3)[:, p]

    # W0 as fp8 pair blocks [128, 2, 128] per chunk; pair row 1 is zero
    # (contraction is only I=128 deep).
    w0t = np.zeros((128, NK, 2, 128), np.float32)
    w0t[:, :, 0, :] = (W0s.T * float(2 ** SW0)).reshape(128, NK, 128)
    assert np.abs(w0t).max() < 235.0
    w0t = np.ascontiguousarray(w0t.reshape(128, NK * 256)).astype(_f8)

    w1t = _pack_pair_f8(W1s.T, _NP_L, 128, [128 * m for m in range(NK)], 2 ** SW)
    w2t = _pack_pair_f8(W2s.T, _NP_L, 128, [128 * m for m in range(NK)], 2 ** SW)
    w3t = _pack_pair_f8(W3s.T, _NP_4, 512, [512 * q for q in range(NQ)], 2 ** SW)

    # biases for ACT-relu chunks (fp32, pre-scaled to the fp8 activation
    # domain); also fp8-packed rows for the bias matmuls of Pool/DVE chunks.
    b0r = np.ascontiguousarray(b0s.reshape(NK, 128).T).astype(np.float32) * float(2 ** SA)
    b1r = np.ascontiguousarray(b1s.reshape(NK, 128).T).astype(np.float32) * float(2 ** SA)
    b2r = np.ascontiguousarray(b2s.reshape(NK, 128).T).astype(np.float32) * float(2 ** SA)
    bm = np.zeros((2, 3 * H), np.float32)
    # b0 range is 1/sqrt(128) so it packs at 2^(SB-2); its matmul const is
    # then 2^(SX+SW0-(SB-2)) = 2^5 = the same 32.0 as layers 1/2.
    bm[0, 0 * H:1 * H] = b0s * float(2 ** (SB - 2))
    bm[0, 1 * H:2 * H] = b1s * float(2 ** SB)
    bm[0, 2 * H:3 * H] = b2s * float(2 ** SB)
    assert np.abs(bm).max() < 235.0
    bmf8 = bm.reshape(1, -1).astype(_f8)
    b3m = np.zeros((2, I * K), np.float32)
    b3m[0] = b3 * float(2 ** SB)
    assert np.abs(b3m).max() < 235.0
    b3f8 = b3m.reshape(1, -1).astype(_f8)

    common = dict(w0t=w0t, w1t=w1t, w2t=w2t, w3t=w3t,
                  b0r=b0r, b1r=b1r, b2r=b2r, bmf8=bmf8, b3f8=b3f8)
    in_maps = []
    for c in range(NCORES):
        m = dict(common)
        xc = np.asarray(x[c * BL:(c + 1) * BL], np.float32)
        # x_d feeds only the logp dt stage; pre-scale by sqrt(0.5) so the
        # downstream square already carries the 0.5 factor, and permute
        # positions to unit order (unit w: chunks w and 7-w) so the dt STT
        # sees one contiguous 32-position block per unit.
        xs = xc * np.float32(math.sqrt(0.5))
        xperm = np.empty((BL, I), np.float32)
        for w in range(4):
            xperm[:, 32 * w:32 * w + 16] = xs[:, 16 * w:16 * w + 16]
            xperm[:, 32 * w + 16:32 * w + 32] = xs[:, 112 - 16 * w:128 - 16 * w]
        m["x"] = np.ascontiguousarray(xperm)
        # x transposed + fp8 pair layout [128, 2, BL] (pair row 1 zero)
        xt = np.zeros((128, 2, BL), np.float32)
        xt[:, 0, :] = xc.T * float(2 ** SX)
        assert np.abs(xt).max() < 235.0
        m["xt8"] = np.ascontiguousarray(xt.reshape(128, 2 * BL)).astype(_f8)
        in_maps.append(m)
    return in_maps


_NC_CACHE = {}


def _build_nc(loop_reps=1):
    import concourse.bacc as bacc
    import concourse.tile as tile
    import concourse.mybir as mybir
    from contextlib import ExitStack

    f32 = mybir.dt.float32
    f16 = mybir.dt.float16
    bf16 = mybir.dt.bfloat16
    f8 = mybir.dt.float8e4
    AF = mybir.ActivationFunctionType
    ALU = mybir.AluOpType
    AX = mybir.AxisListType
    DR = mybir.MatmulPerfMode.DoubleRow

    nc = bacc.Bacc("TRN2")
    x_d = nc.declare_dram_parameter("x", [BL, I], f32, isOutput=False)
    xt_d = nc.declare_dram_parameter("xt8", [128, 2 * BL], f8, isOutput=False)
    w0_d = nc.declare_dram_parameter("w0t", [128, NK * 256], f8, isOutput=False)
    w1_d = nc.declare_dram_parameter("w1t", [128, _OFF1[-1]], f8, isOutput=False)
    w2_d = nc.declare_dram_parameter("w2t", [128, _OFF1[-1]], f8, isOutput=False)
    w3_d = nc.declare_dram_parameter("w3t", [128, _OFF3[-1]], f8, isOutput=False)
    b0_d = nc.declare_dram_parameter("b0r", [128, NK], f32, isOutput=False)
    b1_d = nc.declare_dram_parameter("b1r", [128, NK], f32, isOutput=False)
    b2_d = nc.declare_dram_parameter("b2r", [128, NK], f32, isOutput=False)
    bm_d = nc.declare_dram_parameter("bmf8", [1, 2 * 3 * H], f8, isOutput=False)
    b3_d = nc.declare_dram_parameter("b3f8", [1, 2 * I * K], f8, isOutput=False)
    out_d = nc.declare_dram_parameter("out", [128, NG], f32, isOutput=True)

    with ExitStack() as ctx:
        tc = ctx.enter_context(tile.TileContext(nc))
        consts = ctx.enter_context(tc.tile_pool(name="consts", bufs=1))
        a13p = ctx.enter_context(tc.tile_pool(name="a13p", bufs=NK // 2))
        a2p = ctx.enter_context(tc.tile_pool(name="a2p", bufs=NK // 2))
        wpool = ctx.enter_context(tc.tile_pool(name="wl", bufs=3))
        w3pool = ctx.enter_context(tc.tile_pool(name="w3", bufs=36))
        pallpool = ctx.enter_context(tc.tile_pool(name="pallp", bufs=4))
        ltmp = ctx.enter_context(tc.tile_pool(name="ltmp", bufs=6))
        chpool = ctx.enter_context(tc.tile_pool(name="ch", bufs=4))

        if loop_reps > 1:
            ctx.enter_context(tc.For_i(0, loop_reps, 1))

        # ---- constants ----
        xf = consts.tile([128, NG, I], f32)         # x[p, g, i] = x[g*128+p, i]
        nc.sync.dma_start(out=xf, in_=x_d[:, :].rearrange("(g p) i -> p g i", p=128))
        xt = consts.tile([128, 2, BL], f8)          # fp8 pair-packed x.T
        nc.sync.dma_start(out=xt, in_=xt_d[:, :].rearrange("p (two b) -> p two b", two=2))
        w0sb = consts.tile([128, NK, 2, 128], f8)
        nc.sync.dma_start(out=w0sb, in_=w0_d[:, :].rearrange(
            "p (m two c) -> p m two c", m=NK, two=2))
        b0sb = consts.tile([128, NK], f32)
        nc.sync.dma_start(out=b0sb, in_=b0_d[:, :])
        b1sb = consts.tile([128, NK], f32)
        nc.sync.dma_start(out=b1sb, in_=b1_d[:, :])
        b2sb = consts.tile([128, NK], f32)
        nc.sync.dma_start(out=b2sb, in_=b2_d[:, :])
        bmsb = consts.tile([1, 2, 3 * H], f8)
        nc.sync.dma_start(out=bmsb, in_=bm_d[:, :].rearrange(
            "p (two c) -> p two c", two=2))
        b3sb = consts.tile([1, 2, I * K], f8)
        nc.sync.dma_start(out=b3sb, in_=b3_d[:, :].rearrange(
            "p (two c) -> p two c", two=2))
        # constant moving rows for the bias matmuls: row0 = 2^(SW+SA-SB) or
        # 2^(SX+SW0+SA-SB), row1 = 0
        bcl = consts.tile([1, 2, 512], f8)
        nc.vector.memset(bcl[0:1, 0, :], float(2 ** (SW + SA - SB)))
        nc.vector.memset(bcl[0:1, 1, :], 0.0)
        # stationary for the L4 bias: [1, 2, 128] = 2^(SW+SA-SB), 0
        b3st = consts.tile([1, 2, 128], f8)
        nc.vector.memset(b3st[0:1, 0, :], float(2 ** (SW + SA - SB)))
        nc.vector.memset(b3st[0:1, 1, :], 0.0)
        czero = consts.tile([128, 1], f32)
        nc.vector.memset(czero, 0.0)
        nc.const_aps.aps[(f32, 0.0)] = czero[:, :]
        cnegc = consts.tile([128, 1], f32)
        nc.vector.memset(cnegc, float(-C0 + LNC))
        logs = consts.tile([128, NG, 2, 16], f32)
        nc.vector.memset(logs, 0.0)

        SDESC = float(2.0 ** (-(SW + SA)))   # PSUM holds theta * 2^(SW+SA)

        def relu_emit(m, ps, bsb, Aout, eng, scale):
            """PSUM -> fp8 A tile: relu + rescale on the assigned engine."""
            dst = Aout[m // 2][:, m % 2, :]
            if eng == 'A':
                nc.scalar.activation(dst, ps[:, :], AF.Relu,
                                     bias=bsb[:, m:m + 1], scale=scale)
            elif eng == 'P':
                nc.gpsimd.tensor_scalar(out=dst, in0=ps[:, :], scalar1=scale,
                                        scalar2=0.0, op0=ALU.mult, op1=ALU.max)
            else:
                nc.vector.tensor_scalar(out=dst, in0=ps[:, :], scalar1=scale,
                                        scalar2=0.0, op0=ALU.mult, op1=ALU.max)

        # ---- layers 1-3 in their own PSUM scope (released before L4) ----
        with tc.tile_pool(name="psA", bufs=3, space="PSUM") as psA:
            A1 = [a13p.tile([128, 2, BL], f8, tag="a13", name=f"a1_{kp}")
                  for kp in range(NK // 2)]
            for m in range(NK):
                eng = RELU_ENG[m]
                ps = psA.tile([128, 1024], f32, tag="psA", name=f"ps1_{m}")
                lhs = w0sb[:, m, :, :]
                for n in range(2):
                    half = ps[:, n * 512:(n + 1) * 512]
                    nc.tensor.matmul(half, lhs, xt[:, :, n * 512:(n + 1) * 512],
                                     start=True, stop=(eng == 'A'), perf_mode=DR)
                    if eng != 'A':
                        # bias lands in PSUM via a K=2 fp8 DoubleRow matmul:
                        # stationary bias row (2^SB), moving const 2^(SX+SW0+SA-SB)
                        nc.tensor.matmul(
                            half, bmsb[0:1, :, m * 128:(m + 1) * 128],
                            bcl[0:1, :, :], start=False, stop=True, perf_mode=DR)
                relu_emit(m, ps, b0sb, A1, eng, float(2 ** (SA - SX - SW0)))

            def hidden_layer(w_dram, Ain, bsb, pool, tagp):
                Aout = [pool.tile([128, 2, BL], f8, tag=tagp,
                                  name=f"a{tagp}_{kp}") for kp in range(NK // 2)]
                for m in range(NK):
                    eng = RELU_ENG[m]
                    npair = _NP_L[m]
                    wt = wpool.tile([128, npair * 256], f8, tag="wl",
                                    name=f"w{tagp}_{m}")
                    nc.sync.dma_start(out=wt, in_=w_dram[:, _OFF1[m]:_OFF1[m + 1]])
                    ps = psA.tile([128, 1024], f32, tag="psA", name=f"ps{tagp}_{m}")
                    for kp in range(npair):
                        lhs = wt[:, kp * 256:(kp + 1) * 256].rearrange(
                            "p (two m) -> p two m", two=2)
                        last = (kp == npair - 1) and eng == 'A'
                        nc.tensor.matmul(ps[:, 0:512], lhs,
                                         Ain[kp][:, :, 0:512],
                                         start=(kp == 0), stop=last, perf_mode=DR)
                        nc.tensor.matmul(ps[:, 512:1024], lhs,
                                         Ain[kp][:, :, 512:1024],
                                         start=(kp == 0), stop=last, perf_mode=DR)
                    if eng != 'A':
                        loff = H if tagp == "a2" else 2 * H
                        bslice = bmsb[0:1, :, loff + m * 128:loff + (m + 1) * 128]
                        for n in range(2):
                            nc.tensor.matmul(ps[:, n * 512:(n + 1) * 512], bslice,
                                             bcl[0:1, :, :],
                                             start=False, stop=True, perf_mode=DR)
                    relu_emit(m, ps, bsb, Aout, eng, float(2.0 ** (-SW)))
                return Aout

            A2 = hidden_layer(w1_d, A1, b1sb, a2p, "a2")
            A3 = hidden_layer(w2_d, A2, b2sb, a13p, "a13")

        # ---- layer 4 + logp + paired bidirectional chain (fp16) ----
        PP = []
        for w in range(4):
            pp = pallpool.tile([128, 16, NG, 2, 16], f16, tag="pall",
                               name=f"pp_{w}")
            PP.append(pp)

        state = {"u": None, "n": 0, "r": None}

        def pair_step(p):
            w, li = divmod(p, 16)
            tmp = chpool.tile([128, NG, 2, 4, 4], f16, tag="tmp", name=f"tmp{p}")
            Pv = PP[w][:, li, :, :, :].rearrange("p g f (o i) -> p g f o i", o=4)
            ubc = state["u"][:, :, :, None, :].broadcast_to([128, NG, 2, 4, 4])
            nc.vector.tensor_mul(tmp[:, :, :, :, :], ubc, Pv)
            unew = chpool.tile([128, NG, 2, 4], f16, tag="u", name=f"u{p}")
            with nc.allow_low_precision(reason="fp16 chain, renormed"):
                nc.vector.tensor_reduce(unew[:, :, :, :], tmp[:, :, :, :, :],
                                        axis=AX.X, op=ALU.add)
            if state["r"] is not None:
                # lagged renorm scale from the previous renorm point
                us = chpool.tile([128, NG, 2, 4], f16, tag="u", name=f"us{p}")
                nc.vector.tensor_mul(
                    us[:, :, :, :], unew[:, :, :, :],
                    state["r"][:, :, :, None].broadcast_to([128, NG, 2, 4]))
                state["r"] = None
                unew = us
            state["u"] = unew
            state["n"] += 1
            if state["n"] % 4 == 0 and state["n"] < 64:
                idx = state["n"] // 4 - 1
                m16 = chpool.tile([128, NG, 2], f32, tag="m16", name=f"m16_{p}")
                nc.vector.tensor_reduce(m16[:, :, :], unew[:, :, :, :],
                                        axis=AX.X, op=ALU.max)
                nc.scalar.activation(logs[:, :, :, idx], m16[:, :, :], AF.Ln)
                r16 = chpool.tile([128, NG, 2], f32, tag="r16", name=f"r16_{p}")
                nc.vector.reciprocal(r16[:, :, :], m16[:, :, :])
                state["r"] = r16

        def emit_chain_after(emit_idx):
            if emit_idx == 1:
                u0 = chpool.tile([128, NG, 2, 4], f16, tag="u", name="u_init")
                nc.vector.tensor_copy(
                    u0[:, :, 0, :],
                    PP[0][:, 0, :, 0, :].rearrange(
                        "p g (j k) -> p g j k", j=4)[:, :, :, 0])
                nc.vector.tensor_copy(
                    u0[:, :, 1, :],
                    PP[0][:, 0, :, 1, :].rearrange(
                        "p g (k j) -> p g k j", k=4)[:, :, :, 0])
                state["u"] = u0
                for p in range(1, 16):
                    pair_step(p)
            elif emit_idx in (3, 5, 7):
                w = (emit_idx - 1) // 2
                for p in range(16 * w, 16 * (w + 1)):
                    pair_step(p)

        def logp_unit_pair(ps, qa, qb, g):
            # z-form: z = -alpha - 0.5*t^2 computed with DVE only, then
            # P = exp(z - C0 + ln8) written by ACT straight into the PP lanes.
            psv = ps[:, :].rearrange("p (i t e) -> p i t e", t=2, e=16)
            mu_ap = psv[:, :, 0, :]              # [128, 32, 16]
            al_ap = psv[:, :, 1, :]
            et = ltmp.tile([128, 32, 16], f32, tag="et", name=f"et{qa}_{g}")
            nc.scalar.activation(et[:, :, :], al_ap, AF.Exp, scale=-SDESC)
            dt_ = ltmp.tile([128, 32, 16], f32, tag="dt", name=f"dt{qa}_{g}")
            xbc = xf[:, g, qa * 32:(qa + 1) * 32][:, :, None] \
                .broadcast_to([128, 32, 16])
            nc.vector.scalar_tensor_tensor(
                out=dt_[:, :, :], in0=mu_ap[:, :, :],
                scalar=float(SDESC * math.sqrt(0.5)), in1=xbc,
                op0=ALU.mult, op1=ALU.subtract)
            tt_ = ltmp.tile([128, 32, 16], f32, tag="tt", name=f"tt{qa}_{g}")
            nc.vector.tensor_mul(tt_[:, :, :], dt_[:, :, :], et[:, :, :])
            sq = ltmp.tile([128, 32, 16], f32, tag="sq", name=f"sq{qa}_{g}")
            nc.vector.tensor_mul(sq[:, :, :], tt_[:, :, :], tt_[:, :, :])
            z = ltmp.tile([128, 32, 16], f32, tag="z", name=f"z{qa}_{g}")
            nc.vector.scalar_tensor_tensor(
                out=z[:, :, :], in0=al_ap, scalar=-SDESC, in1=sq[:, :, :],
                op0=ALU.mult, op1=ALU.subtract)
            # left lane (qa): store transposed (j-major), fp16, pre-scaled x8
            nc.scalar.activation(
                PP[qa][:, :, g, 0, :].rearrange("p li (j k) -> p li k j", j=4),
                z[:, 0:16, :].rearrange("p li (k j) -> p li k j", k=4),
                AF.Exp, bias=cnegc[:, :])
            # right lane (qb): natural k-major, li reversed
            nc.scalar.activation(
                PP[7 - qb][:, ::-1, g, 1, :].rearrange(
                    "p li (k j) -> p li k j", k=4),
                z[:, 16:32, :].rearrange("p li (k j) -> p li k j", k=4),
                AF.Exp, bias=cnegc[:, :])

        w3tiles = {}
        for qc in range(NQ):
            for kp in range(_NP_4[qc]):
                w3t_ = w3pool.tile([128, 2, 512], f8, tag="w3",
                                   name=f"w3_{qc}_{kp}")
                nc.sync.dma_start(
                    out=w3t_,
                    in_=w3_d[:, _OFF3[qc] + kp * 1024:
                             _OFF3[qc] + (kp + 1) * 1024].rearrange(
                                 "p (two n) -> p two n", two=2))
                w3tiles[(qc, kp)] = w3t_

        with tc.tile_pool(name="psB", bufs=4, space="PSUM") as psB:
            for w, (qa, qb) in enumerate([(0, 7), (1, 6), (2, 5), (3, 4)]):
                npa, npb = _NP_4[qa], _NP_4[qb]
                wta = [w3tiles[(qa, kp)] for kp in range(npa)]
                wtb = [w3tiles[(qb, kp)] for kp in range(npb)]
                for g in range(NG):
                    ps = psB.tile([128, 1024], f32, tag="psB", name=f"ps4_{qa}_{g}")
                    for kp in range(npb):
                        lhs = A3[kp][:, :, g * 128:(g + 1) * 128]
                        if kp < npa:
                            nc.tensor.matmul(
                                ps[:, 0:512], lhs, wta[kp][:, :, :],
                                start=(kp == 0), stop=False, perf_mode=DR)
                        nc.tensor.matmul(
                            ps[:, 512:1024], lhs, wtb[kp][:, :, :],
                            start=(kp == 0), stop=False, perf_mode=DR)
                    nc.tensor.matmul(ps[:, 0:512], b3st[0:1, :, :],
                                     b3sb[0:1, :, qa * 512:(qa + 1) * 512],
                                     start=False, stop=True, perf_mode=DR)
                    nc.tensor.matmul(ps[:, 512:1024], b3st[0:1, :, :],
                                     b3sb[0:1, :, qb * 512:(qb + 1) * 512],
                                     start=False, stop=True, perf_mode=DR)
                    logp_unit_pair(ps, qa, qb, g)
                emit_chain_after(2 * w + 1)

            # ---- finalize: res = ln(sum_k uL[k]*uR[k]) + sum(logs) - 128*LNC
            u2 = state["u"]
            r_pend = state["r"]
            tmp2 = chpool.tile([128, NG, 4], f32, tag="tmp2")
            nc.vector.tensor_mul(tmp2[:, :, :], u2[:, :, 0, :], u2[:, :, 1, :])
            tot = chpool.tile([128, NG], f32, tag="tot")
            nc.vector.tensor_reduce(tot[:, :], tmp2[:, :, :], axis=AX.X, op=ALU.add)
            if r_pend is not None:
                # fold the pending lagged scale (left*right lanes) into tot
                rr = chpool.tile([128, NG], f32, tag="rr")
                nc.vector.tensor_mul(rr[:, :], r_pend[:, :, 0], r_pend[:, :, 1])
                tot2 = chpool.tile([128, NG], f32, tag="tot2")
                nc.vector.tensor_mul(tot2[:, :], tot[:, :], rr[:, :])
                tot = tot2
            lgt = chpool.tile([128, NG], f32, tag="lgt")
            nc.scalar.activation(lgt[:, :], tot[:, :], AF.Ln)
            ssum = chpool.tile([128, NG], f32, tag="ssum")
            nc.vector.tensor_reduce(ssum[:, :], logs[:, :, :, :], axis=AX.XY,
                                    op=ALU.add)
            res = chpool.tile([128, NG], f32, tag="res")
            nc.vector.scalar_tensor_tensor(
                out=res[:, :], in0=lgt[:, :], scalar=float(-128.0 * LNC),
                in1=ssum[:, :], op0=ALU.add, op1=ALU.add)
            nc.sync.dma_start(out=out_d[:, :], in_=res[:, :])

    import concourse.hw_specs as hw_specs
    _orig_tables = hw_specs.get_activation_tables(nc.m.arch)
    _pinned = {
        name: (funcs if name == "natural_log_exp_and_others" else set())
        for name, funcs in _orig_tables.items()
    }
    _orig_fn = hw_specs.get_activation_tables
    try:
        hw_specs.get_activation_tables = lambda arch: _pinned
        import concourse.bacc as bacc_mod
        if hasattr(bacc_mod, "get_activation_tables"):
            bacc_mod.get_activation_tables = lambda arch: _pinned
        nc.compile()
    finally:
        hw_specs.get_activation_tables = _orig_fn
        if hasattr(bacc_mod, "get_activation_tables"):
            bacc_mod.get_activation_tables = _orig_fn
    return nc


def _get_nc(loop_reps=1):
    key = ("nc", loop_reps)
    if key not in _NC_CACHE:
        _NC_CACHE[key] = _build_nc(loop_reps)
    return _NC_CACHE[key]


def run_on_hw(in_maps, trace=False):
    from concourse.bass_utils import run_bass_kernel_spmd
    nc = _get_nc()
    return run_bass_kernel_spmd(nc, in_maps, list(range(NCORES)), trace=trace)


def kernel(**inputs):
    inputs = {k: np.asarray(v, dtype=np.float32) for k, v in inputs.items()}
    in_maps = _prep_inputs(
        inputs["x"], inputs["W0"], inputs["b0"], inputs["W1"], inputs["b1"],
        inputs["W2"], inputs["b2"], inputs["W3"], inputs["b3"])
    res = run_on_hw(in_maps)
    out = np.empty((B,), np.float32)
    for c in range(NCORES):
        out[c * BL:(c + 1) * BL] = res.results[c]["out"].T.reshape(BL)
    return out.reshape(B, 1, 1)


# revision 11
# speedup vs baseline: 1.0123x; 1.0123x over previous
"""Trainium2 Bass kernel for the DART masked-MLP + log-semiring chain model.

Computes, for B=8192 samples distributed over 8 NeuronCores (1024 each):
  h1 = relu(x @ (m0*W0).T + b0)
  h2 = relu(h1 @ (m1*W1).T + b1)
  h3 = relu(h2 @ (m2*W2).T + b2)
  theta = (h3 @ (m3*W3).T + b3) -> (B, 128, 2, 4, 4) = (mu, alpha)
  logp  = -0.5*((x - mu)*exp(-alpha))**2 - alpha - 0.5*log(2pi) - log(4)
  out   = logexpmm(first, logexpmm(chain(inner), last))   # (B, 1, 1)

Device strategy (per core):
  - MADE masks premultiplied into the weights host-side; hidden units sorted
    by MADE degree so the masked weights are block lower triangular and ~47%
    of K-chunks are skipped.
  - ALL four matmul layers run fp8-e4m3 DoubleRow (256-wide contraction per
    instruction, 0.5 PE cycles per output column).  x is pre-transposed and
    pre-packed to the fp8 pair layout host-side, so no on-device transpose.
  - ALL biases are added in PSUM by tiny fp8 DoubleRow matmuls (stationary
    carries the bias row at 2^13, moving a constant 2^(S-13) row), freeing
    the relu from the per-partition-bias ACT so it can run on any engine.
  - The PSUM->fp8 relu+rescale is load-balanced across the ACT, Pool and DVE
    engines (single tensor_scalar op: max(psum*2^-S, 0)).
  - logp pipeline per 1024-wide theta pair-chunk: exp on ACT, the (mu-x)
    STT on DVE, tt/sq/P-store multiplies on Pool.  P is stored in fp16,
    pre-scaled by 8 (P<=0.106 for this model, so 8*P<1; the product picks up
    8^128 which is subtracted as a compile-time constant at the end).
  - The 126-step log-semiring chain runs in fp16 in the linear domain,
    folded simultaneously from both ends as paired steps in single DVE ops
    (fp16 gets the DVE 2x perf mode on the multiply).  A max-renormalization
    every 4 pair-steps keeps fp16 in range; its scale is applied one step
    late so the max/reciprocal run off the critical path.  L4 output chunks
    are produced outside-in so the chain overlaps the matmuls.
"""

import math

import numpy as np
import ml_dtypes

I = 128          # input size / positions
H = 2048         # hidden
A = 4            # alpha_dim
K = 2 * A * A    # 32 theta entries per position
B = 8192
NCORES = 8
BL = B // NCORES          # 1024 samples per core
NG = BL // 128            # 8 sample groups of 128
NK = H // 128             # 16 hidden chunks
NQ = (I * K) // 512       # 8 output q-chunks (512 wide = 16 positions)
C0 = 0.5 * math.log(2.0 * math.pi) + math.log(4.0)
SW = 13                   # weight scale 2^SW for fp8 (max |w|*2^13 ~ 181 < 240)
SA = 5                    # activation scale 2^SA for fp8 (max h*32 ~ 80 < 240)
SX = 5                    # x scale 2^SX for fp8 (max |x|*32 ~ 144 < 240)
SW0 = 11                  # W0 scale 2^SW0 (max |w0|*2^11 ~ 181 < 240)
SB = 13                   # bias fp8 scale 2^SB (max |b|*2^13 ~ 181 < 240)
LNC = 3.0 * math.log(2.0)   # P pre-scale ln(8); P*8 <= 0.85 < 1
# per-layer relu engine split: 'A' = ACT, 'D' = DVE.  (Pool/GPSIMD cannot
# read PSUM on TRN2, so it only gets SBUF-side work.)
RELU_ENG = ['A'] * 16

_bf16 = ml_dtypes.bfloat16
_f8 = ml_dtypes.float8_e4m3


def _make_meta():
    """Degree sort + triangular chunk metadata (static)."""
    hdeg = np.arange(H) % (I - 1)
    perm = np.argsort(hdeg, kind="stable")
    sdeg = hdeg[perm]
    km_l = []
    for m in range(NK):
        dhi = sdeg[128 * m + 127]
        km_l.append(max(k for k in range(NK) if sdeg[128 * k] <= dhi))
    km_4 = []
    for qc in range(NQ):
        dhi = 16 * qc + 15 - 1
        cands = [k for k in range(NK) if sdeg[128 * k] <= dhi]
        km_4.append(max(cands) if cands else -1)
    return perm, km_l, km_4


_PERM, _KM_L, _KM_4 = _make_meta()
_NP_L = [k // 2 + 1 for k in _KM_L]     # fp8 DoubleRow pair-chunks (256-wide K)
_NP_4 = [k // 2 + 1 for k in _KM_4]
_OFF1 = np.cumsum([0] + [p * 256 for p in _NP_L]).tolist()
_OFF3 = np.cumsum([0] + [p * 1024 for p in _NP_4]).tolist()


def _pack_pair_f8(WT, npairs, out_w, col_starts, scale):
    """fp8 DoubleRow blocks [128, 2, out_w] per (block, pair)."""
    cols = []
    for blk, (np_, c0) in enumerate(zip(npairs, col_starts)):
        for kp in range(np_):
            blkdat = np.stack(
                [WT[256 * kp + 128 * par:256 * kp + 128 * (par + 1),
                    c0:c0 + out_w] for par in range(2)], axis=1)
            cols.append(blkdat.reshape(128, 2 * out_w))
    arr = np.concatenate(cols, axis=1) * float(scale)
    assert np.abs(arr).max() < 235.0, np.abs(arr).max()
    return np.ascontiguousarray(arr).astype(_f8)


def _prep_inputs(x, W0, b0, W1, b1, W2, b2, W3, b3):
    """Host-side: premask, degree-sort, pack and cast the weights."""
    inp = np.arange(I)
    degrees = [inp] + [np.arange(H) % (I - 1) for _ in range(3)] + [np.arange(I) - 1]
    masks = [
        (d1[:, None] >= d0[None, :]).astype(np.float32)
        for d0, d1 in zip(degrees[:-1], degrees[1:])
    ]
    masks[-1] = np.repeat(masks[-1], K, axis=0)

    p = _PERM
    W0s = (masks[0] * W0)[p]
    b0s = b0[p]
    W1s = (masks[1] * W1)[p][:, p]
    b1s = b1[p]
    W2s = (masks[2] * W2)[p][:, p]
    b2s = b2[p]
    W3s = (masks[3] * W3)[:, p]

    # W0 as fp8 pair blocks [128, 2, 128] per chunk; pair row 1 is zero
    # (contraction is only I=128 deep).
    w0t = np.zeros((128, NK, 2, 128), np.float32)
    w0t[:, :, 0, :] = (W0s.T * float(2 ** SW0)).reshape(128, NK, 128)
    assert np.abs(w0t).max() < 235.0
    w0t = np.ascontiguousarray(w0t.reshape(128, NK * 256)).astype(_f8)

    w1t = _pack_pair_f8(W1s.T, _NP_L, 128, [128 * m for m in range(NK)], 2 ** SW)
    w2t = _pack_pair_f8(W2s.T, _NP_L, 128, [128 * m for m in range(NK)], 2 ** SW)
    w3t = _pack_pair_f8(W3s.T, _NP_4, 512, [512 * q for q in range(NQ)], 2 ** SW)

    # biases for ACT-relu chunks (fp32, pre-scaled to the fp8 activation
    # domain); also fp8-packed rows for the bias matmuls of Pool/DVE chunks.
    b0r = np.ascontiguousarray(b0s.reshape(NK, 128).T).astype(np.float32) * float(2 ** SA)
    b1r = np.ascontiguousarray(b1s.reshape(NK, 128).T).astype(np.float32) * float(2 ** SA)
    b2r = np.ascontiguousarray(b2s.reshape(NK, 128).T).astype(np.float32) * float(2 ** SA)
    bm = np.zeros((2, 3 * H), np.float32)
    # b0 range is 1/sqrt(128) so it packs at 2^(SB-2); its matmul const is
    # then 2^(SX+SW0-(SB-2)) = 2^5 = the same 32.0 as layers 1/2.
    bm[0, 0 * H:1 * H] = b0s * float(2 ** (SB - 2))
    bm[0, 1 * H:2 * H] = b1s * float(2 ** SB)
    bm[0, 2 * H:3 * H] = b2s * float(2 ** SB)
    assert np.abs(bm).max() < 235.0
    bmf8 = bm.reshape(1, -1).astype(_f8)
    b3m = np.zeros((2, I * K), np.float32)
    b3m[0] = b3 * float(2 ** SB)
    assert np.abs(b3m).max() < 235.0
    b3f8 = b3m.reshape(1, -1).astype(_f8)

    common = dict(w0t=w0t, w1t=w1t, w2t=w2t, w3t=w3t,
                  b0r=b0r, b1r=b1r, b2r=b2r, bmf8=bmf8, b3f8=b3f8)
    in_maps = []
    for c in range(NCORES):
        m = dict(common)
        xc = np.asarray(x[c * BL:(c + 1) * BL], np.float32)
        # x_d feeds only the logp dt stage; pre-scale by sqrt(0.5) so the
        # downstream square already carries the 0.5 factor, and permute
        # positions to unit order (unit w: chunks w and 7-w) so the dt STT
        # sees one contiguous 32-position block per unit.
        xs = xc * np.float32(math.sqrt(0.5))
        xperm = np.empty((BL, I), np.float32)
        for w in range(4):
            xperm[:, 32 * w:32 * w + 16] = xs[:, 16 * w:16 * w + 16]
            xperm[:, 32 * w + 16:32 * w + 32] = xs[:, 112 - 16 * w:128 - 16 * w]
        m["x"] = np.ascontiguousarray(xperm)
        # x transposed + fp8 pair layout [128, 2, BL] (pair row 1 zero)
        xt = np.zeros((128, 2, BL), np.float32)
        xt[:, 0, :] = xc.T * float(2 ** SX)
        assert np.abs(xt).max() < 235.0
        m["xt8"] = np.ascontiguousarray(xt.reshape(128, 2 * BL)).astype(_f8)
        in_maps.append(m)
    return in_maps


_NC_CACHE = {}


def _build_nc(loop_reps=1):
    import concourse.bacc as bacc
    import concourse.tile as tile
    import concourse.mybir as mybir
    from contextlib import ExitStack

    f32 = mybir.dt.float32
    f16 = mybir.dt.float16
    bf16 = mybir.dt.bfloat16
    f8 = mybir.dt.float8e4
    AF = mybir.ActivationFunctionType
    ALU = mybir.AluOpType
    AX = mybir.AxisListType
    DR = mybir.MatmulPerfMode.DoubleRow

    nc = bacc.Bacc("TRN2")
    x_d = nc.declare_dram_parameter("x", [BL, I], f32, isOutput=False)
    xt_d = nc.declare_dram_parameter("xt8", [128, 2 * BL], f8, isOutput=False)
    w0_d = nc.declare_dram_parameter("w0t", [128, NK * 256], f8, isOutput=False)
    w1_d = nc.declare_dram_parameter("w1t", [128, _OFF1[-1]], f8, isOutput=False)
    w2_d = nc.declare_dram_parameter("w2t", [128, _OFF1[-1]], f8, isOutput=False)
    w3_d = nc.declare_dram_parameter("w3t", [128, _OFF3[-1]], f8, isOutput=False)
    b0_d = nc.declare_dram_parameter("b0r", [128, NK], f32, isOutput=False)
    b1_d = nc.declare_dram_parameter("b1r", [128, NK], f32, isOutput=False)
    b2_d = nc.declare_dram_parameter("b2r", [128, NK], f32, isOutput=False)
    bm_d = nc.declare_dram_parameter("bmf8", [1, 2 * 3 * H], f8, isOutput=False)
    b3_d = nc.declare_dram_parameter("b3f8", [1, 2 * I * K], f8, isOutput=False)
    out_d = nc.declare_dram_parameter("out", [128, NG], f32, isOutput=True)

    with ExitStack() as ctx:
        tc = ctx.enter_context(tile.TileContext(nc))
        consts = ctx.enter_context(tc.tile_pool(name="consts", bufs=1))
        a13p = ctx.enter_context(tc.tile_pool(name="a13p", bufs=NK // 2))
        a2p = ctx.enter_context(tc.tile_pool(name="a2p", bufs=NK // 2))
        wpool = ctx.enter_context(tc.tile_pool(name="wl", bufs=3))
        w3pool = ctx.enter_context(tc.tile_pool(name="w3", bufs=1))
        pallpool = ctx.enter_context(tc.tile_pool(name="pallp", bufs=4))
        ltmp = ctx.enter_context(tc.tile_pool(name="ltmp", bufs=6))
        chpool = ctx.enter_context(tc.tile_pool(name="ch", bufs=4))

        if loop_reps > 1:
            ctx.enter_context(tc.For_i(0, loop_reps, 1))

        # ---- constants ----
        xf = consts.tile([128, NG, I], f32)         # x[p, g, i] = x[g*128+p, i]
        nc.sync.dma_start(out=xf, in_=x_d[:, :].rearrange("(g p) i -> p g i", p=128))
        xt = consts.tile([128, 2, BL], f8)          # fp8 pair-packed x.T
        nc.sync.dma_start(out=xt, in_=xt_d[:, :].rearrange("p (two b) -> p two b", two=2))
        w0sb = consts.tile([128, NK, 2, 128], f8)
        nc.sync.dma_start(out=w0sb, in_=w0_d[:, :].rearrange(
            "p (m two c) -> p m two c", m=NK, two=2))
        b0sb = consts.tile([128, NK], f32)
        nc.sync.dma_start(out=b0sb, in_=b0_d[:, :])
        b1sb = consts.tile([128, NK], f32)
        nc.sync.dma_start(out=b1sb, in_=b1_d[:, :])
        b2sb = consts.tile([128, NK], f32)
        nc.sync.dma_start(out=b2sb, in_=b2_d[:, :])
        bmsb = consts.tile([1, 2, 3 * H], f8)
        nc.sync.dma_start(out=bmsb, in_=bm_d[:, :].rearrange(
            "p (two c) -> p two c", two=2))
        b3sb = consts.tile([1, 2, I * K], f8)
        nc.sync.dma_start(out=b3sb, in_=b3_d[:, :].rearrange(
            "p (two c) -> p two c", two=2))
        # constant moving rows for the bias matmuls: row0 = 2^(SW+SA-SB) or
        # 2^(SX+SW0+SA-SB), row1 = 0
        bcl = consts.tile([1, 2, 512], f8)
        nc.vector.memset(bcl[0:1, 0, :], float(2 ** (SW + SA - SB)))
        nc.vector.memset(bcl[0:1, 1, :], 0.0)
        # stationary for the L4 bias: [1, 2, 128] = 2^(SW+SA-SB), 0
        b3st = consts.tile([1, 2, 128], f8)
        nc.vector.memset(b3st[0:1, 0, :], float(2 ** (SW + SA - SB)))
        nc.vector.memset(b3st[0:1, 1, :], 0.0)
        czero = consts.tile([128, 1], f32)
        nc.vector.memset(czero, 0.0)
        nc.const_aps.aps[(f32, 0.0)] = czero[:, :]
        cnegc = consts.tile([128, 1], f32)
        nc.vector.memset(cnegc, float(-C0 + LNC))
        logs = consts.tile([128, NG, 2, 16], f32)
        nc.vector.memset(logs, 0.0)

        SDESC = float(2.0 ** (-(SW + SA)))   # PSUM holds theta * 2^(SW+SA)

        def relu_emit(m, ps, bsb, Aout, eng, scale):
            """PSUM -> fp8 A tile: relu + rescale on the assigned engine."""
            dst = Aout[m // 2][:, m % 2, :]
            if eng == 'A':
                nc.scalar.activation(dst, ps[:, :], AF.Relu,
                                     bias=bsb[:, m:m + 1], scale=scale)
            elif eng == 'P':
                nc.gpsimd.tensor_scalar(out=dst, in0=ps[:, :], scalar1=scale,
                                        scalar2=0.0, op0=ALU.mult, op1=ALU.max)
            else:
                nc.vector.tensor_scalar(out=dst, in0=ps[:, :], scalar1=scale,
                                        scalar2=0.0, op0=ALU.mult, op1=ALU.max)

        # ---- layers 1-3 in their own PSUM scope (released before L4) ----
        with tc.tile_pool(name="psA", bufs=3, space="PSUM") as psA:
            A1 = [a13p.tile([128, 2, BL], f8, tag="a13", name=f"a1_{kp}")
                  for kp in range(NK // 2)]
            for m in range(NK):
                eng = RELU_ENG[m]
                ps = psA.tile([128, 1024], f32, tag="psA", name=f"ps1_{m}")
                lhs = w0sb[:, m, :, :]
                for n in range(2):
                    half = ps[:, n * 512:(n + 1) * 512]
                    nc.tensor.matmul(half, lhs, xt[:, :, n * 512:(n + 1) * 512],
                                     start=True, stop=(eng == 'A'), perf_mode=DR)
                    if eng != 'A':
                        # bias lands in PSUM via a K=2 fp8 DoubleRow matmul:
                        # stationary bias row (2^SB), moving const 2^(SX+SW0+SA-SB)
                        nc.tensor.matmul(
                            half, bmsb[0:1, :, m * 128:(m + 1) * 128],
                            bcl[0:1, :, :], start=False, stop=True, perf_mode=DR)
                relu_emit(m, ps, b0sb, A1, eng, float(2 ** (SA - SX - SW0)))

            def hidden_layer(w_dram, Ain, bsb, pool, tagp):
                Aout = [pool.tile([128, 2, BL], f8, tag=tagp,
                                  name=f"a{tagp}_{kp}") for kp in range(NK // 2)]
                for m in range(NK):
                    eng = RELU_ENG[m]
                    npair = _NP_L[m]
                    wt = wpool.tile([128, npair * 256], f8, tag="wl",
                                    name=f"w{tagp}_{m}")
                    nc.sync.dma_start(out=wt, in_=w_dram[:, _OFF1[m]:_OFF1[m + 1]])
                    ps = psA.tile([128, 1024], f32, tag="psA", name=f"ps{tagp}_{m}")
                    for kp in range(npair):
                        lhs = wt[:, kp * 256:(kp + 1) * 256].rearrange(
                            "p (two m) -> p two m", two=2)
                        last = (kp == npair - 1) and eng == 'A'
                        nc.tensor.matmul(ps[:, 0:512], lhs,
                                         Ain[kp][:, :, 0:512],
                                         start=(kp == 0), stop=last, perf_mode=DR)
                        nc.tensor.matmul(ps[:, 512:1024], lhs,
                                         Ain[kp][:, :, 512:1024],
                                         start=(kp == 0), stop=last, perf_mode=DR)
                    if eng != 'A':
                        loff = H if tagp == "a2" else 2 * H
                        bslice = bmsb[0:1, :, loff + m * 128:loff + (m + 1) * 128]
                        for n in range(2):
                            nc.tensor.matmul(ps[:, n * 512:(n + 1) * 512], bslice,
                                             bcl[0:1, :, :],
                                             start=False, stop=True, perf_mode=DR)
                    relu_emit(m, ps, bsb, Aout, eng, float(2.0 ** (-SW)))
                return Aout

            A2 = hidden_layer(w1_d, A1, b1sb, a2p, "a2")
            A3 = hidden_layer(w2_d, A2, b2sb, a13p, "a13")

        # ---- layer 4 + logp + paired bidirectional chain (fp16) ----
        PP = []
        for w in range(4):
            pp = pallpool.tile([128, 16, NG, 2, 16], f16, tag="pall",
                               name=f"pp_{w}")
            PP.append(pp)

        state = {"u": None, "n": 0, "r": None}

        def pair_step(p):
            w, li = divmod(p, 16)
            tmp = chpool.tile([128, NG, 2, 4, 4], f16, tag="tmp", name=f"tmp{p}")
            Pv = PP[w][:, li, :, :, :].rearrange("p g f (o i) -> p g f o i", o=4)
            ubc = state["u"][:, :, :, None, :].broadcast_to([128, NG, 2, 4, 4])
            nc.vector.tensor_mul(tmp[:, :, :, :, :], ubc, Pv)
            unew = chpool.tile([128, NG, 2, 4], f16, tag="u", name=f"u{p}")
            with nc.allow_low_precision(reason="fp16 chain, renormed"):
                nc.vector.tensor_reduce(unew[:, :, :, :], tmp[:, :, :, :, :],
                                        axis=AX.X, op=ALU.add)
            if state["r"] is not None:
                # lagged renorm scale from the previous renorm point
                us = chpool.tile([128, NG, 2, 4], f16, tag="u", name=f"us{p}")
                nc.vector.tensor_mul(
                    us[:, :, :, :], unew[:, :, :, :],
                    state["r"][:, :, :, None].broadcast_to([128, NG, 2, 4]))
                state["r"] = None
                unew = us
            state["u"] = unew
            state["n"] += 1
            if state["n"] % 4 == 0 and state["n"] < 64:
                idx = state["n"] // 4 - 1
                m16 = chpool.tile([128, NG, 2], f32, tag="m16", name=f"m16_{p}")
                nc.vector.tensor_reduce(m16[:, :, :], unew[:, :, :, :],
                                        axis=AX.X, op=ALU.max)
                nc.scalar.activation(logs[:, :, :, idx], m16[:, :, :], AF.Ln)
                r16 = chpool.tile([128, NG, 2], f32, tag="r16", name=f"r16_{p}")
                nc.vector.reciprocal(r16[:, :, :], m16[:, :, :])
                state["r"] = r16

        def emit_chain_after(emit_idx):
            if emit_idx == 1:
                u0 = chpool.tile([128, NG, 2, 4], f16, tag="u", name="u_init")
                nc.vector.tensor_copy(
                    u0[:, :, 0, :],
                    PP[0][:, 0, :, 0, :].rearrange(
                        "p g (j k) -> p g j k", j=4)[:, :, :, 0])
                nc.vector.tensor_copy(
                    u0[:, :, 1, :],
                    PP[0][:, 0, :, 1, :].rearrange(
                        "p g (k j) -> p g k j", k=4)[:, :, :, 0])
                state["u"] = u0
                for p in range(1, 16):
                    pair_step(p)
            elif emit_idx in (3, 5, 7):
                w = (emit_idx - 1) // 2
                for p in range(16 * w, 16 * (w + 1)):
                    pair_step(p)

        def logp_unit_pair(ps, qa, qb, g):
            # z-form: z = -alpha - 0.5*t^2 computed with DVE only, then
            # P = exp(z - C0 + ln8) written by ACT straight into the PP lanes.
            psv = ps[:, :].rearrange("p (i t e) -> p i t e", t=2, e=16)
            mu_ap = psv[:, :, 0, :]              # [128, 32, 16]
            al_ap = psv[:, :, 1, :]
            et = ltmp.tile([128, 32, 16], f32, tag="et", name=f"et{qa}_{g}")
            nc.scalar.activation(et[:, :, :], al_ap, AF.Exp, scale=-SDESC)
            dt_ = ltmp.tile([128, 32, 16], f32, tag="dt", name=f"dt{qa}_{g}")
            xbc = xf[:, g, qa * 32:(qa + 1) * 32][:, :, None] \
                .broadcast_to([128, 32, 16])
            nc.vector.scalar_tensor_tensor(
                out=dt_[:, :, :], in0=mu_ap[:, :, :],
                scalar=float(SDESC * math.sqrt(0.5)), in1=xbc,
                op0=ALU.mult, op1=ALU.subtract)
            tt_ = ltmp.tile([128, 32, 16], f32, tag="tt", name=f"tt{qa}_{g}")
            nc.vector.tensor_mul(tt_[:, :, :], dt_[:, :, :], et[:, :, :])
            sq = ltmp.tile([128, 32, 16], f32, tag="sq", name=f"sq{qa}_{g}")
            nc.vector.tensor_mul(sq[:, :, :], tt_[:, :, :], tt_[:, :, :])
            z = ltmp.tile([128, 32, 16], f32, tag="z", name=f"z{qa}_{g}")
            nc.vector.scalar_tensor_tensor(
                out=z[:, :, :], in0=al_ap, scalar=-SDESC, in1=sq[:, :, :],
                op0=ALU.mult, op1=ALU.subtract)
            # left lane (qa): store transposed (j-major), fp16, pre-scaled x8
            nc.scalar.activation(
                PP[qa][:, :, g, 0, :].rearrange("p li (j k) -> p li k j", j=4),
                z[:, 0:16, :].rearrange("p li (k j) -> p li k j", k=4),
                AF.Exp, bias=cnegc[:, :])
            # right lane (qb): natural k-major, li reversed
            nc.scalar.activation(
                PP[7 - qb][:, ::-1, g, 1, :].rearrange(
                    "p li (k j) -> p li k j", k=4),
                z[:, 16:32, :].rearrange("p li (k j) -> p li k j", k=4),
                AF.Exp, bias=cnegc[:, :])

        w3flat = w3pool.tile([128, _OFF3[-1]], f8, tag="w3", name="w3flat")
        nc.sync.dma_start(out=w3flat, in_=w3_d[:, :])
        w3tiles = {}
        for qc in range(NQ):
            for kp in range(_NP_4[qc]):
                w3tiles[(qc, kp)] = w3flat[
                    :, _OFF3[qc] + kp * 1024:_OFF3[qc] + (kp + 1) * 1024
                ].rearrange("p (two n) -> p two n", two=2)

        with tc.tile_pool(name="psB", bufs=4, space="PSUM") as psB:
            for w, (qa, qb) in enumerate([(0, 7), (1, 6), (2, 5), (3, 4)]):
                npa, npb = _NP_4[qa], _NP_4[qb]
                wta = [w3tiles[(qa, kp)] for kp in range(npa)]
                wtb = [w3tiles[(qb, kp)] for kp in range(npb)]
                for g in range(NG):
                    ps = psB.tile([128, 1024], f32, tag="psB", name=f"ps4_{qa}_{g}")
                    for kp in range(npb):
                        lhs = A3[kp][:, :, g * 128:(g + 1) * 128]
                        if kp < npa:
                            nc.tensor.matmul(
                                ps[:, 0:512], lhs, wta[kp][:, :, :],
                                start=(kp == 0), stop=False, perf_mode=DR)
                        nc.tensor.matmul(
                            ps[:, 512:1024], lhs, wtb[kp][:, :, :],
                            start=(kp == 0), stop=False, perf_mode=DR)
                    nc.tensor.matmul(ps[:, 0:512], b3st[0:1, :, :],
                                     b3sb[0:1, :, qa * 512:(qa + 1) * 512],
                                     start=False, stop=True, perf_mode=DR)
                    nc.tensor.matmul(ps[:, 512:1024], b3st[0:1, :, :],
                                     b3sb[0:1, :, qb * 512:(qb + 1) * 512],
                                     start=False, stop=True, perf_mode=DR)
                    logp_unit_pair(ps, qa, qb, g)
                emit_chain_after(2 * w + 1)

            # ---- finalize: res = ln(sum_k uL[k]*uR[k]) + sum(logs) - 128*LNC
            u2 = state["u"]
            r_pend = state["r"]
            tmp2 = chpool.tile([128, NG, 4], f32, tag="tmp2")
            nc.vector.tensor_mul(tmp2[:, :, :], u2[:, :, 0, :], u2[:, :, 1, :])
            tot = chpool.tile([128, NG], f32, tag="tot")
            nc.vector.tensor_reduce(tot[:, :], tmp2[:, :, :], axis=AX.X, op=ALU.add)
            if r_pend is not None:
                # fold the pending lagged scale (left*right lanes) into tot
                rr = chpool.tile([128, NG], f32, tag="rr")
                nc.vector.tensor_mul(rr[:, :], r_pend[:, :, 0], r_pend[:, :, 1])
                tot2 = chpool.tile([128, NG], f32, tag="tot2")
                nc.vector.tensor_mul(tot2[:, :], tot[:, :], rr[:, :])
                tot = tot2
            lgt = chpool.tile([128, NG], f32, tag="lgt")
            nc.scalar.activation(lgt[:, :], tot[:, :], AF.Ln)
            ssum = chpool.tile([128, NG], f32, tag="ssum")
            nc.vector.tensor_reduce(ssum[:, :], logs[:, :, :, :], axis=AX.XY,
                                    op=ALU.add)
            res = chpool.tile([128, NG], f32, tag="res")
            nc.vector.scalar_tensor_tensor(
                out=res[:, :], in0=lgt[:, :], scalar=float(-128.0 * LNC),
                in1=ssum[:, :], op0=ALU.add, op1=ALU.add)
            nc.sync.dma_start(out=out_d[:, :], in_=res[:, :])

    import concourse.hw_specs as hw_specs
    _orig_tables = hw_specs.get_activation_tables(nc.m.arch)
    _pinned = {
        name: (funcs if name == "natural_log_exp_and_others" else set())
        for name, funcs in _orig_tables.items()
    }
    _orig_fn = hw_specs.get_activation_tables
    try:
        hw_specs.get_activation_tables = lambda arch: _pinned
        import concourse.bacc as bacc_mod
        if hasattr(bacc_mod, "get_activation_tables"):
            bacc_mod.get_activation_tables = lambda arch: _pinned
        nc.compile()
    finally:
        hw_specs.get_activation_tables = _orig_fn
        if hasattr(bacc_mod, "get_activation_tables"):
            bacc_mod.get_activation_tables = _orig_fn
    return nc


def _get_nc(loop_reps=1):
    key = ("nc", loop_reps)
    if key not in _NC_CACHE:
        _NC_CACHE[key] = _build_nc(loop_reps)
    return _NC_CACHE[key]


def run_on_hw(in_maps, trace=False):
    from concourse.bass_utils import run_bass_kernel_spmd
    nc = _get_nc()
    return run_bass_kernel_spmd(nc, in_maps, list(range(NCORES)), trace=trace)


def kernel(**inputs):
    inputs = {k: np.asarray(v, dtype=np.float32) for k, v in inputs.items()}
    in_maps = _prep_inputs(
        inputs["x"], inputs["W0"], inputs["b0"], inputs["W1"], inputs["b1"],
        inputs["W2"], inputs["b2"], inputs["W3"], inputs["b3"])
    res = run_on_hw(in_maps)
    out = np.empty((B,), np.float32)
    for c in range(NCORES):
        out[c * BL:(c + 1) * BL] = res.results[c]["out"].T.reshape(BL)
    return out.reshape(B, 1, 1)


# revision 12
# speedup vs baseline: 1.1616x; 1.1475x over previous
"""Trainium2 Bass kernel for the DART masked-MLP + log-semiring chain model.

Computes, for B=8192 samples distributed over 8 NeuronCores (1024 each):
  h1 = relu(x @ (m0*W0).T + b0)
  h2 = relu(h1 @ (m1*W1).T + b1)
  h3 = relu(h2 @ (m2*W2).T + b2)
  theta = (h3 @ (m3*W3).T + b3) -> (B, 128, 2, 4, 4) = (mu, alpha)
  logp  = -0.5*((x - mu)*exp(-alpha))**2 - alpha - 0.5*log(2pi) - log(4)
  out   = logexpmm(first, logexpmm(chain(inner), last))   # (B, 1, 1)

Device strategy (per core):
  - MADE masks premultiplied into the weights host-side; hidden units sorted
    by MADE degree so the masked weights are block lower triangular and ~47%
    of K-chunks are skipped.
  - ALL four matmul layers run fp8-e4m3 DoubleRow (256-wide contraction per
    instruction, 0.5 PE cycles per output column).  x is pre-transposed and
    pre-packed to the fp8 pair layout host-side, so no on-device transpose.
  - ALL biases are added in PSUM by tiny fp8 DoubleRow matmuls (stationary
    carries the bias row at 2^13, moving a constant 2^(S-13) row), freeing
    the relu from the per-partition-bias ACT so it can run on any engine.
  - The PSUM->fp8 relu+rescale is load-balanced across the ACT, Pool and DVE
    engines (single tensor_scalar op: max(psum*2^-S, 0)).
  - logp pipeline per 1024-wide theta pair-chunk: exp on ACT, the (mu-x)
    STT on DVE, tt/sq/P-store multiplies on Pool.  P is stored in fp16,
    pre-scaled by 8 (P<=0.106 for this model, so 8*P<1; the product picks up
    8^128 which is subtracted as a compile-time constant at the end).
  - The 126-step log-semiring chain runs in fp16 in the linear domain,
    folded simultaneously from both ends as paired steps in single DVE ops
    (fp16 gets the DVE 2x perf mode on the multiply).  A max-renormalization
    every 4 pair-steps keeps fp16 in range; its scale is applied one step
    late so the max/reciprocal run off the critical path.  L4 output chunks
    are produced outside-in so the chain overlaps the matmuls.
"""

import math

import numpy as np
import ml_dtypes

I = 128          # input size / positions
H = 2048         # hidden
A = 4            # alpha_dim
K = 2 * A * A    # 32 theta entries per position
B = 8192
NCORES = 8
BL = B // NCORES          # 1024 samples per core
NG = BL // 128            # 8 sample groups of 128
NK = H // 128             # 16 hidden chunks
NQ = (I * K) // 512       # 8 output q-chunks (512 wide = 16 positions)
C0 = 0.5 * math.log(2.0 * math.pi) + math.log(4.0)
SW = 13                   # weight scale 2^SW for fp8 (max |w|*2^13 ~ 181 < 240)
SA = 5                    # activation scale 2^SA for fp8 (max h*32 ~ 80 < 240)
SX = 5                    # x scale 2^SX for fp8 (max |x|*32 ~ 144 < 240)
SW0 = 11                  # W0 scale 2^SW0 (max |w0|*2^11 ~ 181 < 240)
SB = 13                   # bias fp8 scale 2^SB (max |b|*2^13 ~ 181 < 240)
LNC = 3.0 * math.log(2.0)   # P pre-scale ln(8); P*8 <= 0.85 < 1
# per-layer relu engine split: 'A' = ACT, 'D' = DVE.  (Pool/GPSIMD cannot
# read PSUM on TRN2, so it only gets SBUF-side work.)
RELU_ENG = ['A'] * 16

_bf16 = ml_dtypes.bfloat16
_f8 = ml_dtypes.float8_e4m3


def _make_meta():
    """Degree sort + triangular chunk metadata (static)."""
    hdeg = np.arange(H) % (I - 1)
    perm = np.argsort(hdeg, kind="stable")
    sdeg = hdeg[perm]
    km_l = []
    for m in range(NK):
        dhi = sdeg[128 * m + 127]
        km_l.append(max(k for k in range(NK) if sdeg[128 * k] <= dhi))
    km_4 = []
    for qc in range(NQ):
        dhi = 16 * qc + 15 - 1
        cands = [k for k in range(NK) if sdeg[128 * k] <= dhi]
        km_4.append(max(cands) if cands else -1)
    return perm, km_l, km_4


_PERM, _KM_L, _KM_4 = _make_meta()
_NP_L = [k // 2 + 1 for k in _KM_L]     # fp8 DoubleRow pair-chunks (256-wide K)
_NP_4 = [k // 2 + 1 for k in _KM_4]
_OFF1 = np.cumsum([0] + [p * 256 for p in _NP_L]).tolist()
_OFF3 = np.cumsum([0] + [p * 1024 for p in _NP_4]).tolist()


def _pack_pair_f8(WT, npairs, out_w, col_starts, scale):
    """fp8 DoubleRow blocks [128, 2, out_w] per (block, pair)."""
    cols = []
    for blk, (np_, c0) in enumerate(zip(npairs, col_starts)):
        for kp in range(np_):
            blkdat = np.stack(
                [WT[256 * kp + 128 * par:256 * kp + 128 * (par + 1),
                    c0:c0 + out_w] for par in range(2)], axis=1)
            cols.append(blkdat.reshape(128, 2 * out_w))
    arr = np.concatenate(cols, axis=1) * float(scale)
    assert np.abs(arr).max() < 235.0, np.abs(arr).max()
    return np.ascontiguousarray(arr).astype(_f8)


def _prep_inputs(x, W0, b0, W1, b1, W2, b2, W3, b3):
    """Host-side: premask, degree-sort, pack and cast the weights."""
    inp = np.arange(I)
    degrees = [inp] + [np.arange(H) % (I - 1) for _ in range(3)] + [np.arange(I) - 1]
    masks = [
        (d1[:, None] >= d0[None, :]).astype(np.float32)
        for d0, d1 in zip(degrees[:-1], degrees[1:])
    ]
    masks[-1] = np.repeat(masks[-1], K, axis=0)

    p = _PERM
    W0s = (masks[0] * W0)[p]
    b0s = b0[p]
    W1s = (masks[1] * W1)[p][:, p]
    b1s = b1[p]
    W2s = (masks[2] * W2)[p][:, p]
    b2s = b2[p]
    W3s = (masks[3] * W3)[:, p]

    # W0 as fp8 pair blocks [128, 2, 128] per chunk; pair row 1 is zero
    # (contraction is only I=128 deep).
    w0t = np.zeros((128, NK, 2, 128), np.float32)
    w0t[:, :, 0, :] = (W0s.T * float(2 ** SW0)).reshape(128, NK, 128)
    assert np.abs(w0t).max() < 235.0
    w0t = np.ascontiguousarray(w0t.reshape(128, NK * 256)).astype(_f8)

    w1t = _pack_pair_f8(W1s.T, _NP_L, 128, [128 * m for m in range(NK)], 2 ** SW)
    w2t = _pack_pair_f8(W2s.T, _NP_L, 128, [128 * m for m in range(NK)], 2 ** SW)
    w3t = _pack_pair_f8(W3s.T, _NP_4, 512, [512 * q for q in range(NQ)], 2 ** SW)

    # biases for ACT-relu chunks (fp32, pre-scaled to the fp8 activation
    # domain); also fp8-packed rows for the bias matmuls of Pool/DVE chunks.
    b0r = np.ascontiguousarray(b0s.reshape(NK, 128).T).astype(np.float32) * float(2 ** SA)
    b1r = np.ascontiguousarray(b1s.reshape(NK, 128).T).astype(np.float32) * float(2 ** SA)
    b2r = np.ascontiguousarray(b2s.reshape(NK, 128).T).astype(np.float32) * float(2 ** SA)
    bm = np.zeros((2, 3 * H), np.float32)
    # b0 range is 1/sqrt(128) so it packs at 2^(SB-2); its matmul const is
    # then 2^(SX+SW0-(SB-2)) = 2^5 = the same 32.0 as layers 1/2.
    bm[0, 0 * H:1 * H] = b0s * float(2 ** (SB - 2))
    bm[0, 1 * H:2 * H] = b1s * float(2 ** SB)
    bm[0, 2 * H:3 * H] = b2s * float(2 ** SB)
    assert np.abs(bm).max() < 235.0
    bmf8 = bm.reshape(1, -1).astype(_f8)
    b3m = np.zeros((2, I * K), np.float32)
    b3m[0] = b3 * float(2 ** SB)
    assert np.abs(b3m).max() < 235.0
    b3f8 = b3m.reshape(1, -1).astype(_f8)

    common = dict(w0t=w0t, w1t=w1t, w2t=w2t, w3t=w3t,
                  b0r=b0r, b1r=b1r, b2r=b2r, b3f8=b3f8)
    in_maps = []
    for c in range(NCORES):
        m = dict(common)
        xc = np.asarray(x[c * BL:(c + 1) * BL], np.float32)
        # x_d feeds only the logp dt stage; pre-scale by sqrt(0.5) so the
        # downstream square already carries the 0.5 factor, and permute
        # positions to unit order (unit w: chunks w and 7-w) so the dt STT
        # sees one contiguous 32-position block per unit.
        xs = xc * np.float32(math.sqrt(0.5))
        xperm = np.empty((BL, I), np.float32)
        for w in range(4):
            xperm[:, 32 * w:32 * w + 16] = xs[:, 16 * w:16 * w + 16]
            xperm[:, 32 * w + 16:32 * w + 32] = xs[:, 112 - 16 * w:128 - 16 * w]
        xh = np.empty((128, NG * I), np.float32)
        for g in range(NG):
            xh[:, g * I:(g + 1) * I] = xperm[g * 128:(g + 1) * 128, :]
        m["x"] = np.ascontiguousarray(xh)
        # x transposed + fp8 pair layout [128, 2, BL] (pair row 1 zero)
        xt = np.zeros((128, 2, BL), np.float32)
        xt[:, 0, :] = xc.T * float(2 ** SX)
        assert np.abs(xt).max() < 235.0
        m["xt8"] = np.ascontiguousarray(xt.reshape(128, 2 * BL)).astype(_f8)
        in_maps.append(m)
    return in_maps


_NC_CACHE = {}


def _build_nc(loop_reps=1):
    import concourse.bacc as bacc
    import concourse.tile as tile
    import concourse.mybir as mybir
    from contextlib import ExitStack

    f32 = mybir.dt.float32
    f16 = mybir.dt.float16
    bf16 = mybir.dt.bfloat16
    f8 = mybir.dt.float8e4
    AF = mybir.ActivationFunctionType
    ALU = mybir.AluOpType
    AX = mybir.AxisListType
    DR = mybir.MatmulPerfMode.DoubleRow

    nc = bacc.Bacc("TRN2")
    x_d = nc.declare_dram_parameter("x", [128, NG * I], f32, isOutput=False)
    xt_d = nc.declare_dram_parameter("xt8", [128, 2 * BL], f8, isOutput=False)
    w0_d = nc.declare_dram_parameter("w0t", [128, NK * 256], f8, isOutput=False)
    w1_d = nc.declare_dram_parameter("w1t", [128, _OFF1[-1]], f8, isOutput=False)
    w2_d = nc.declare_dram_parameter("w2t", [128, _OFF1[-1]], f8, isOutput=False)
    w3_d = nc.declare_dram_parameter("w3t", [128, _OFF3[-1]], f8, isOutput=False)
    b0_d = nc.declare_dram_parameter("b0r", [128, NK], f32, isOutput=False)
    b1_d = nc.declare_dram_parameter("b1r", [128, NK], f32, isOutput=False)
    b2_d = nc.declare_dram_parameter("b2r", [128, NK], f32, isOutput=False)
    b3_d = nc.declare_dram_parameter("b3f8", [1, 2 * I * K], f8, isOutput=False)
    out_d = nc.declare_dram_parameter("out", [128, NG], f32, isOutput=True)

    with ExitStack() as ctx:
        tc = ctx.enter_context(tile.TileContext(nc))
        consts = ctx.enter_context(tc.tile_pool(name="consts", bufs=1))
        a13p = ctx.enter_context(tc.tile_pool(name="a13p", bufs=NK // 2))
        a2p = ctx.enter_context(tc.tile_pool(name="a2p", bufs=NK // 2))
        wpool = ctx.enter_context(tc.tile_pool(name="wl", bufs=3))
        w3pool = ctx.enter_context(tc.tile_pool(name="w3", bufs=1))
        pallpool = ctx.enter_context(tc.tile_pool(name="pallp", bufs=4))
        ltmp = ctx.enter_context(tc.tile_pool(name="ltmp", bufs=6))
        chpool = ctx.enter_context(tc.tile_pool(name="ch", bufs=4))

        if loop_reps > 1:
            ctx.enter_context(tc.For_i(0, loop_reps, 1))

        # ---- constants ----
        xf = consts.tile([128, NG, I], f32)         # x[p, g, i] = x[g*128+p, i]
        nc.sync.dma_start(out=xf, in_=x_d[:, :].rearrange("p (g i) -> p g i", g=NG))
        xt = consts.tile([128, 2, BL], f8)          # fp8 pair-packed x.T
        nc.sync.dma_start(out=xt, in_=xt_d[:, :].rearrange("p (two b) -> p two b", two=2))
        w0sb = consts.tile([128, NK, 2, 128], f8)
        nc.sync.dma_start(out=w0sb, in_=w0_d[:, :].rearrange(
            "p (m two c) -> p m two c", m=NK, two=2))
        b0sb = consts.tile([128, NK], f32)
        nc.sync.dma_start(out=b0sb, in_=b0_d[:, :])
        b1sb = consts.tile([128, NK], f32)
        nc.sync.dma_start(out=b1sb, in_=b1_d[:, :])
        b2sb = consts.tile([128, NK], f32)
        nc.sync.dma_start(out=b2sb, in_=b2_d[:, :])
        b3sb = consts.tile([1, 2, I * K], f8)
        nc.sync.dma_start(out=b3sb, in_=b3_d[:, :].rearrange(
            "p (two c) -> p two c", two=2))
        # constant moving rows for the bias matmuls: row0 = 2^(SW+SA-SB) or
        # 2^(SX+SW0+SA-SB), row1 = 0
        # stationary for the L4 bias: [1, 2, 128] = 2^(SW+SA-SB), 0
        b3st = consts.tile([1, 2, 128], f8)
        nc.vector.memset(b3st[0:1, 0, :], float(2 ** (SW + SA - SB)))
        nc.vector.memset(b3st[0:1, 1, :], 0.0)
        czero = consts.tile([128, 1], f32)
        nc.vector.memset(czero, 0.0)
        nc.const_aps.aps[(f32, 0.0)] = czero[:, :]
        cnegc = consts.tile([128, 1], f32)
        nc.vector.memset(cnegc, float(-C0 + LNC))
        logs = consts.tile([128, NG, 2, 16], f32)
        nc.vector.memset(logs, 0.0)

        SDESC = float(2.0 ** (-(SW + SA)))   # PSUM holds theta * 2^(SW+SA)

        def relu_emit(m, ps, bsb, Aout, eng, scale):
            """PSUM -> fp8 A tile: relu + rescale on the assigned engine."""
            dst = Aout[m // 2][:, m % 2, :]
            if eng == 'A':
                nc.scalar.activation(dst, ps[:, :], AF.Relu,
                                     bias=bsb[:, m:m + 1], scale=scale)
            elif eng == 'P':
                nc.gpsimd.tensor_scalar(out=dst, in0=ps[:, :], scalar1=scale,
                                        scalar2=0.0, op0=ALU.mult, op1=ALU.max)
            else:
                nc.vector.tensor_scalar(out=dst, in0=ps[:, :], scalar1=scale,
                                        scalar2=0.0, op0=ALU.mult, op1=ALU.max)

        # ---- layers 1-3 in their own PSUM scope (released before L4) ----
        with tc.tile_pool(name="psA", bufs=3, space="PSUM") as psA:
            A1 = [a13p.tile([128, 2, BL], f8, tag="a13", name=f"a1_{kp}")
                  for kp in range(NK // 2)]
            for m in range(NK):
                eng = RELU_ENG[m]
                ps = psA.tile([128, 1024], f32, tag="psA", name=f"ps1_{m}")
                lhs = w0sb[:, m, :, :]
                for n in range(2):
                    half = ps[:, n * 512:(n + 1) * 512]
                    nc.tensor.matmul(half, lhs, xt[:, :, n * 512:(n + 1) * 512],
                                     start=True, stop=(eng == 'A'), perf_mode=DR)
                    if eng != 'A':
                        # bias lands in PSUM via a K=2 fp8 DoubleRow matmul:
                        # stationary bias row (2^SB), moving const 2^(SX+SW0+SA-SB)
                        nc.tensor.matmul(
                            half, bmsb[0:1, :, m * 128:(m + 1) * 128],
                            bcl[0:1, :, :], start=False, stop=True, perf_mode=DR)
                relu_emit(m, ps, b0sb, A1, eng, float(2 ** (SA - SX - SW0)))

            def hidden_layer(w_dram, Ain, bsb, pool, tagp):
                Aout = [pool.tile([128, 2, BL], f8, tag=tagp,
                                  name=f"a{tagp}_{kp}") for kp in range(NK // 2)]
                for m in range(NK):
                    eng = RELU_ENG[m]
                    npair = _NP_L[m]
                    wt = wpool.tile([128, npair * 256], f8, tag="wl",
                                    name=f"w{tagp}_{m}")
                    nc.sync.dma_start(out=wt, in_=w_dram[:, _OFF1[m]:_OFF1[m + 1]])
                    ps = psA.tile([128, 1024], f32, tag="psA", name=f"ps{tagp}_{m}")
                    for kp in range(npair):
                        lhs = wt[:, kp * 256:(kp + 1) * 256].rearrange(
                            "p (two m) -> p two m", two=2)
                        last = (kp == npair - 1) and eng == 'A'
                        nc.tensor.matmul(ps[:, 0:512], lhs,
                                         Ain[kp][:, :, 0:512],
                                         start=(kp == 0), stop=last, perf_mode=DR)
                        nc.tensor.matmul(ps[:, 512:1024], lhs,
                                         Ain[kp][:, :, 512:1024],
                                         start=(kp == 0), stop=last, perf_mode=DR)
                    if eng != 'A':
                        loff = H if tagp == "a2" else 2 * H
                        bslice = bmsb[0:1, :, loff + m * 128:loff + (m + 1) * 128]
                        for n in range(2):
                            nc.tensor.matmul(ps[:, n * 512:(n + 1) * 512], bslice,
                                             bcl[0:1, :, :],
                                             start=False, stop=True, perf_mode=DR)
                    relu_emit(m, ps, bsb, Aout, eng, float(2.0 ** (-SW)))
                return Aout

            A2 = hidden_layer(w1_d, A1, b1sb, a2p, "a2")
            A3 = hidden_layer(w2_d, A2, b2sb, a13p, "a13")

        # ---- layer 4 + logp + paired bidirectional chain (fp16) ----
        PP = []
        for w in range(4):
            pp = pallpool.tile([128, 16, NG, 2, 16], f16, tag="pall",
                               name=f"pp_{w}")
            PP.append(pp)

        state = {"u": None, "n": 0, "r": None}

        def pair_step(p):
            w, li = divmod(p, 16)
            tmp = chpool.tile([128, NG, 2, 4, 4], f16, tag="tmp", name=f"tmp{p}")
            Pv = PP[w][:, li, :, :, :].rearrange("p g f (o i) -> p g f o i", o=4)
            ubc = state["u"][:, :, :, None, :].broadcast_to([128, NG, 2, 4, 4])
            nc.vector.tensor_mul(tmp[:, :, :, :, :], ubc, Pv)
            unew = chpool.tile([128, NG, 2, 4], f16, tag="u", name=f"u{p}")
            with nc.allow_low_precision(reason="fp16 chain, renormed"):
                nc.vector.tensor_reduce(unew[:, :, :, :], tmp[:, :, :, :, :],
                                        axis=AX.X, op=ALU.add)
            if state["r"] is not None:
                # lagged renorm scale from the previous renorm point
                us = chpool.tile([128, NG, 2, 4], f16, tag="u", name=f"us{p}")
                nc.vector.tensor_mul(
                    us[:, :, :, :], unew[:, :, :, :],
                    state["r"][:, :, :, None].broadcast_to([128, NG, 2, 4]))
                state["r"] = None
                unew = us
            state["u"] = unew
            state["n"] += 1
            if state["n"] % 4 == 0 and state["n"] < 64:
                idx = state["n"] // 4 - 1
                m16 = chpool.tile([128, NG, 2], f32, tag="m16", name=f"m16_{p}")
                nc.vector.tensor_reduce(m16[:, :, :], unew[:, :, :, :],
                                        axis=AX.X, op=ALU.max)
                nc.scalar.activation(logs[:, :, :, idx], m16[:, :, :], AF.Ln)
                r16 = chpool.tile([128, NG, 2], f32, tag="r16", name=f"r16_{p}")
                nc.vector.reciprocal(r16[:, :, :], m16[:, :, :])
                state["r"] = r16

        def emit_chain_after(emit_idx):
            if emit_idx == 1:
                u0 = chpool.tile([128, NG, 2, 4], f16, tag="u", name="u_init")
                nc.vector.tensor_copy(
                    u0[:, :, 0, :],
                    PP[0][:, 0, :, 0, :].rearrange(
                        "p g (j k) -> p g j k", j=4)[:, :, :, 0])
                nc.vector.tensor_copy(
                    u0[:, :, 1, :],
                    PP[0][:, 0, :, 1, :].rearrange(
                        "p g (k j) -> p g k j", k=4)[:, :, :, 0])
                state["u"] = u0
                for p in range(1, 16):
                    pair_step(p)
            elif emit_idx in (3, 5, 7):
                w = (emit_idx - 1) // 2
                for p in range(16 * w, 16 * (w + 1)):
                    pair_step(p)

        def logp_unit_pair(ps, qa, qb, g):
            # z-form: z = -alpha - 0.5*t^2 computed with DVE only, then
            # P = exp(z - C0 + ln8) written by ACT straight into the PP lanes.
            psv = ps[:, :].rearrange("p (i t e) -> p i t e", t=2, e=16)
            mu_ap = psv[:, :, 0, :]              # [128, 32, 16]
            al_ap = psv[:, :, 1, :]
            et = ltmp.tile([128, 32, 16], f32, tag="et", name=f"et{qa}_{g}")
            nc.scalar.activation(et[:, :, :], al_ap, AF.Exp, scale=-SDESC)
            dt_ = ltmp.tile([128, 32, 16], f32, tag="dt", name=f"dt{qa}_{g}")
            xbc = xf[:, g, qa * 32:(qa + 1) * 32][:, :, None] \
                .broadcast_to([128, 32, 16])
            nc.vector.scalar_tensor_tensor(
                out=dt_[:, :, :], in0=mu_ap[:, :, :],
                scalar=float(SDESC * math.sqrt(0.5)), in1=xbc,
                op0=ALU.mult, op1=ALU.subtract)
            tt_ = ltmp.tile([128, 32, 16], f32, tag="tt", name=f"tt{qa}_{g}")
            nc.vector.tensor_mul(tt_[:, :, :], dt_[:, :, :], et[:, :, :])
            sq = ltmp.tile([128, 32, 16], f32, tag="sq", name=f"sq{qa}_{g}")
            nc.vector.tensor_mul(sq[:, :, :], tt_[:, :, :], tt_[:, :, :])
            z = ltmp.tile([128, 32, 16], f32, tag="z", name=f"z{qa}_{g}")
            nc.vector.scalar_tensor_tensor(
                out=z[:, :, :], in0=al_ap, scalar=-SDESC, in1=sq[:, :, :],
                op0=ALU.mult, op1=ALU.subtract)
            # left lane (qa): store transposed (j-major), fp16, pre-scaled x8
            nc.scalar.activation(
                PP[qa][:, :, g, 0, :].rearrange("p li (j k) -> p li k j", j=4),
                z[:, 0:16, :].rearrange("p li (k j) -> p li k j", k=4),
                AF.Exp, bias=cnegc[:, :])
            # right lane (qb): natural k-major, li reversed
            nc.scalar.activation(
                PP[7 - qb][:, ::-1, g, 1, :].rearrange(
                    "p li (k j) -> p li k j", k=4),
                z[:, 16:32, :].rearrange("p li (k j) -> p li k j", k=4),
                AF.Exp, bias=cnegc[:, :])

        w3flat = w3pool.tile([128, _OFF3[-1]], f8, tag="w3", name="w3flat")
        nc.sync.dma_start(out=w3flat, in_=w3_d[:, :])
        w3tiles = {}
        for qc in range(NQ):
            for kp in range(_NP_4[qc]):
                w3tiles[(qc, kp)] = w3flat[
                    :, _OFF3[qc] + kp * 1024:_OFF3[qc] + (kp + 1) * 1024
                ].rearrange("p (two n) -> p two n", two=2)

        with tc.tile_pool(name="psB", bufs=4, space="PSUM") as psB:
            for w, (qa, qb) in enumerate([(0, 7), (1, 6), (2, 5), (3, 4)]):
                npa, npb = _NP_4[qa], _NP_4[qb]
                wta = [w3tiles[(qa, kp)] for kp in range(npa)]
                wtb = [w3tiles[(qb, kp)] for kp in range(npb)]
                for g in range(NG):
                    ps = psB.tile([128, 1024], f32, tag="psB", name=f"ps4_{qa}_{g}")
                    for kp in range(npb):
                        lhs = A3[kp][:, :, g * 128:(g + 1) * 128]
                        if kp < npa:
                            nc.tensor.matmul(
                                ps[:, 0:512], lhs, wta[kp][:, :, :],
                                start=(kp == 0), stop=False, perf_mode=DR)
                        nc.tensor.matmul(
                            ps[:, 512:1024], lhs, wtb[kp][:, :, :],
                            start=(kp == 0), stop=False, perf_mode=DR)
                    nc.tensor.matmul(ps[:, 0:512], b3st[0:1, :, :],
                                     b3sb[0:1, :, qa * 512:(qa + 1) * 512],
                                     start=False, stop=True, perf_mode=DR)
                    nc.tensor.matmul(ps[:, 512:1024], b3st[0:1, :, :],
                                     b3sb[0:1, :, qb * 512:(qb + 1) * 512],
                                     start=False, stop=True, perf_mode=DR)
                    logp_unit_pair(ps, qa, qb, g)
                emit_chain_after(2 * w + 1)

            # ---- finalize: res = ln(sum_k uL[k]*uR[k]) + sum(logs) - 128*LNC
            u2 = state["u"]
            r_pend = state["r"]
            tmp2 = chpool.tile([128, NG, 4], f32, tag="tmp2")
            nc.vector.tensor_mul(tmp2[:, :, :], u2[:, :, 0, :], u2[:, :, 1, :])
            tot = chpool.tile([128, NG], f32, tag="tot")
            nc.vector.tensor_reduce(tot[:, :], tmp2[:, :, :], axis=AX.X, op=ALU.add)
            if r_pend is not None:
                # fold the pending lagged scale (left*right lanes) into tot
                rr = chpool.tile([128, NG], f32, tag="rr")
                nc.vector.tensor_mul(rr[:, :], r_pend[:, :, 0], r_pend[:, :, 1])
                tot2 = chpool.tile([128, NG], f32, tag="tot2")
                nc.vector.tensor_mul(tot2[:, :], tot[:, :], rr[:, :])
                tot = tot2
            lgt = chpool.tile([128, NG], f32, tag="lgt")
            nc.scalar.activation(lgt[:, :], tot[:, :], AF.Ln)
            ssum = chpool.tile([128, NG], f32, tag="ssum")
            nc.vector.tensor_reduce(ssum[:, :], logs[:, :, :, :], axis=AX.XY,
                                    op=ALU.add)
            res = chpool.tile([128, NG], f32, tag="res")
            nc.vector.scalar_tensor_tensor(
                out=res[:, :], in0=lgt[:, :], scalar=float(-128.0 * LNC),
                in1=ssum[:, :], op0=ALU.add, op1=ALU.add)
            nc.sync.dma_start(out=out_d[:, :], in_=res[:, :])

    import concourse.hw_specs as hw_specs
    _orig_tables = hw_specs.get_activation_tables(nc.m.arch)
    _pinned = {
        name: (funcs if name == "natural_log_exp_and_others" else set())
        for name, funcs in _orig_tables.items()
    }
    _orig_fn = hw_specs.get_activation_tables
    try:
        hw_specs.get_activation_tables = lambda arch: _pinned
        import concourse.bacc as bacc_mod
        if hasattr(bacc_mod, "get_activation_tables"):
            bacc_mod.get_activation_tables = lambda arch: _pinned
        nc.compile()
    finally:
        hw_specs.get_activation_tables = _orig_fn
        if hasattr(bacc_mod, "get_activation_tables"):
            bacc_mod.get_activation_tables = _orig_fn
    return nc


def _get_nc(loop_reps=1):
    key = ("nc", loop_reps)
    if key not in _NC_CACHE:
        _NC_CACHE[key] = _build_nc(loop_reps)
    return _NC_CACHE[key]


def run_on_hw(in_maps, trace=False):
    from concourse.bass_utils import run_bass_kernel_spmd
    nc = _get_nc()
    return run_bass_kernel_spmd(nc, in_maps, list(range(NCORES)), trace=trace)


def kernel(**inputs):
    inputs = {k: np.asarray(v, dtype=np.float32) for k, v in inputs.items()}
    in_maps = _prep_inputs(
        inputs["x"], inputs["W0"], inputs["b0"], inputs["W1"], inputs["b1"],
        inputs["W2"], inputs["b2"], inputs["W3"], inputs["b3"])
    res = run_on_hw(in_maps)
    out = np.empty((B,), np.float32)
    for c in range(NCORES):
        out[c * BL:(c + 1) * BL] = res.results[c]["out"].T.reshape(BL)
    return out.reshape(B, 1, 1)
